# revision 12
# baseline (speedup 1.0000x reference)
"""Trainium2 Bass kernel for nn_CSG_layer (CSG layer: latent-conditioned softmax
mixing + gumbel routing + batched GEMM + tiny MLP side output).

Contract: kernel(**inputs) takes FULL unsharded inputs (as produced by
setup_inputs) and returns the full outputs (y, v_encode). Internally shards
batch dim 32 across 8 NeuronCores (4 batches/core), runs one SPMD Bass
program, and gathers.

Self-contained: only imports environment-provided packages (numpy, concourse).
"""
import os
import numpy as np
from contextlib import ExitStack

import concourse.bass as bass
import concourse.bacc as bacc
import concourse.tile as tile
from concourse import mybir
from concourse.bass_utils import run_bass_kernel_spmd
from concourse.masks import make_identity

F32 = mybir.dt.float32
BF16 = mybir.dt.bfloat16
EPS = float(np.finfo(np.float32).eps)

# Problem shape constants (fixed by the problem spec).
B, P, S_IN, S_OUT, L = 32, 8192, 64, 32, 256
N_CORES = 8
BPC = B // N_CORES          # batches per core = 4
Q = 2 * S_OUT * S_IN        # 4096 flattened (side, o, i)

# Main-GEMM compute dtype: "f32" (exact) or "bf16" (fast).
MM_DTYPE = os.environ.get("CSG_MM_DTYPE", "f32")
# Logits matmul dtype: "f32" or "f32r"
LOGITS_DTYPE = os.environ.get("CSG_LOGITS_DTYPE", "f32")

# Stash of the last BassKernelResults (for test.py profiling access).
LAST_RESULTS = None

_PROGRAM_CACHE = {}


def _build_program(p_exp: float):
    """Build the SPMD Bass program (same on every core)."""
    nc = bacc.Bacc()

    x_mm_dt = BF16 if MM_DTYPE == "bf16" else F32

    # ---- DRAM I/O (per-core shard shapes) ----
    xT_d = nc.dram_tensor("xT", [BPC, S_IN, P], x_mm_dt, kind="ExternalInput")
    kcat_d = nc.dram_tensor("kcat", [L, Q], F32, kind="ExternalInput")
    latT_d = nc.dram_tensor("latT", [L, BPC], F32, kind="ExternalInput")
    guT_d = nc.dram_tensor("guT", [S_IN, 2, S_OUT, BPC], F32, kind="ExternalInput")
    w1_d = nc.dram_tensor("w1", [Q, L], F32, kind="ExternalInput")
    w2_d = nc.dram_tensor("w2", [L, L], F32, kind="ExternalInput")
    b1_d = nc.dram_tensor("b1v", [L], F32, kind="ExternalInput")
    b2_d = nc.dram_tensor("b2v", [L], F32, kind="ExternalInput")
    y_d = nc.dram_tensor("y", [BPC, P, 4 * S_OUT], F32, kind="ExternalOutput")
    ve_d = nc.dram_tensor("ve", [L, BPC], F32, kind="ExternalOutput")

    AF = mybir.ActivationFunctionType
    OP = mybir.AluOpType

    with tile.TileContext(nc) as tc, ExitStack() as ctx:
        consts = ctx.enter_context(tc.tile_pool(name="consts", bufs=1))
        sb = ctx.enter_context(tc.tile_pool(name="sb", bufs=1))

        ident = consts.tile([128, 128], F32)
        make_identity(nc, ident)
        ones64 = consts.tile([64, 1], F32)
        nc.vector.memset(ones64, 1.0)
        eps_sb = consts.tile([128, 1], F32)
        nc.vector.memset(eps_sb, EPS)

        # ---- small input DMAs ----
        lat_sb = sb.tile([128, 2, BPC], F32)
        nc.sync.dma_start(lat_sb, latT_d.rearrange("(c p) b -> p c b", p=128))
        kcat_sb = sb.tile([128, 2, Q], F32)
        nc.sync.dma_start(kcat_sb, kcat_d.rearrange("(c p) q -> p c q", p=128))
        gu_sb = sb.tile([S_IN, 2, S_OUT, BPC], F32)
        nc.sync.dma_start(gu_sb, guT_d[:])
        w1_sb = sb.tile([128, Q // 128, L], F32)
        nc.sync.dma_start(w1_sb, w1_d.rearrange("(c p) j -> p c j", p=128))
        w2_sb = sb.tile([128, 2, L], F32)
        nc.sync.dma_start(w2_sb, w2_d.rearrange("(c p) j -> p c j", p=128))
        b1_sb = sb.tile([128, 2], F32)
        nc.sync.dma_start(b1_sb, b1_d.rearrange("(c p) -> p c", p=128))
        b2_sb = sb.tile([128, 2], F32)
        nc.sync.dma_start(b2_sb, b2_d.rearrange("(c p) -> p c", p=128))

        # ---- P1/P2: logits -> E = exp(logits)  (layout [BPC, (side,o,i)]) ----
        E_sb = sb.tile([BPC, Q], F32)
        if LOGITS_DTYPE == "f32r":
            lat_mm = lat_sb[:].bitcast(mybir.dt.float32r)
            kcat_mm = kcat_sb[:].bitcast(mybir.dt.float32r)
        else:
            lat_mm = lat_sb[:]
            kcat_mm = kcat_sb[:]
        with tc.tile_pool(name="ps_log", bufs=2, space="PSUM") as ps_log:
            for qc in range(4):  # chunks of 1024
                pt_log = ps_log.tile([BPC, 1024], F32)
                for half in range(2):
                    lo = qc * 1024 + half * 512
                    for lc in range(2):
                        nc.tensor.matmul(
                            pt_log[:, half * 512:(half + 1) * 512],
                            lat_mm[:, lc, :],
                            kcat_mm[:, lc, lo:lo + 512],
                            start=(lc == 0),
                            stop=(lc == 1),
                        )
                nc.scalar.activation(
                    E_sb[:, qc * 1024:(qc + 1) * 1024], pt_log[:], AF.Exp
                )

        # S[b, g] = sum_i E ; A = E + EPS*S ; lnA = ln(A) ; V = E/S
        S_sb = sb.tile([BPC, 64], F32)
        nc.vector.tensor_reduce(
            S_sb[:], E_sb[:].rearrange("p (g i) -> p g i", i=S_IN),
            axis=mybir.AxisListType.X, op=OP.add,
        )
        A_sb = sb.tile([BPC, Q], F32)
        nc.vector.scalar_tensor_tensor(
            out=A_sb[:].rearrange("p (g i) -> p g i", i=S_IN),
            in0=S_sb[:, :, None].broadcast_to([BPC, 64, S_IN]),
            scalar=EPS,
            in1=E_sb[:].rearrange("p (g i) -> p g i", i=S_IN),
            op0=OP.mult,
            op1=OP.add,
        )
        lnA_sb = sb.tile([BPC, Q], F32)
        nc.scalar.activation(lnA_sb[:], A_sb[:], AF.Ln)
        recipS = sb.tile([BPC, 64], F32)
        nc.vector.reciprocal(recipS[:], S_sb[:])
        V_sb = sb.tile([BPC, Q], F32)
        nc.vector.tensor_mul(
            V_sb[:].rearrange("p (g i) -> p g i", i=S_IN),
            E_sb[:].rearrange("p (g i) -> p g i", i=S_IN),
            recipS[:, :, None].broadcast_to([BPC, 64, S_IN]),
        )

        # ---- P3: PE transposes ----
        # lnAT[i, side, o, b] ; VT[kpart, kc, b]
        lnAT = sb.tile([S_IN, 2, S_OUT, BPC], F32)
        VT = sb.tile([128, Q // 128, BPC], F32)
        with tc.tile_pool(name="ps_tr", bufs=4, space="PSUM") as ps_tr:
            for g in range(64):  # g = side*32 + o
                pt = ps_tr.tile([128, BPC], F32)
                nc.tensor.transpose(
                    pt[:S_IN, :], lnA_sb[:, g * S_IN:(g + 1) * S_IN],
                    ident[:BPC, :BPC],
                )
                nc.any.tensor_copy(lnAT[:, g // 32, g % 32, :], pt[:S_IN, :])
            for kc in range(Q // 128):
                pt = ps_tr.tile([128, BPC], F32)
                nc.tensor.transpose(
                    pt[:], V_sb[:, kc * 128:(kc + 1) * 128], ident[:BPC, :BPC]
                )
                nc.any.tensor_copy(VT[:, kc, :], pt[:])

            # ---- P4: gumbel chain in T layout [S_IN, (side,o,b)] ----
            NF = 2 * S_OUT * BPC  # 256
            uc = sb.tile([S_IN, NF], F32)
            nc.vector.tensor_scalar_max(uc[:], gu_sb[:].rearrange("p a b c -> p (a b c)"), EPS)
            lnu = sb.tile([S_IN, NF], F32)
            nc.scalar.activation(lnu[:], uc[:], AF.Ln)
            lnB = sb.tile([S_IN, NF], F32)
            nc.scalar.activation(
                lnB[:], lnu[:], AF.Ln, scale=-1.0, bias=eps_sb[:S_IN, :]
            )
            dch = sb.tile([S_IN, NF], F32)
            nc.vector.tensor_sub(
                dch[:], lnAT[:].rearrange("p a b c -> p (a b c)"), lnB[:]
            )
            ngum = sb.tile([S_IN, NF], F32)
            nc.scalar.activation(ngum[:], dch[:], AF.Exp, scale=float(p_exp))

            # ---- P5/P6: group sums over i (partition dim) via ones-matmul ----
            sg_ps = ps_tr.tile([1, NF], F32, tag="sg")
            nc.tensor.matmul(sg_ps[:], ones64[:], ngum[:], start=True, stop=True)
            recipG = sb.tile([1, NF], F32)
            nc.vector.reciprocal(recipG[:], sg_ps[:])
        repG = sb.tile([S_IN, NF], F32)
        nc.gpsimd.partition_broadcast(repG[:], recipG[:])
        maskT = sb.tile([S_IN, 2, S_OUT, BPC], F32)
        nc.vector.tensor_mul(
            maskT[:].rearrange("p a b c -> p (a b c)"),
            ngum[:],
            repG[:],
        )

        # ---- P7: per-batch moving operand M2 [64, (b, 2*S_OUT)] ----
        # cols 0:32 = L+R, 32:64 = L-R
        m2_dt = BF16 if MM_DTYPE == "bf16" else F32
        M2 = sb.tile([S_IN, BPC, 2 * S_OUT], m2_dt)
        for b in range(BPC):
            nc.vector.tensor_add(
                M2[:, b, 0:S_OUT], maskT[:, 0, :, b], maskT[:, 1, :, b]
            )
            nc.vector.tensor_sub(
                M2[:, b, S_OUT:2 * S_OUT], maskT[:, 0, :, b], maskT[:, 1, :, b]
            )

        # ---- P8: main GEMM + clip + y out ----
        # per batch: 4 xa tiles of [64, 2048]; 16 chunks of 128 points each
        XCOLS = 2048
        NCH = XCOLS // 128       # 16 chunks per xa tile
        xpool = ctx.enter_context(tc.tile_pool(name="xpool", bufs=2))
        ypool = ctx.enter_context(tc.tile_pool(name="ypool", bufs=2))
        with tc.tile_pool(name="ps_main", bufs=6, space="PSUM") as ps_main:
            for b in range(BPC):
                for t4 in range(P // XCOLS):
                    xa = xpool.tile([S_IN, XCOLS], x_mm_dt, tag="xa")
                    nc.sync.dma_start(
                        xa[:], xT_d[b, :, t4 * XCOLS:(t4 + 1) * XCOLS]
                    )
                    y_sb = ypool.tile([128, NCH, 4 * S_OUT], F32, tag="ysb")
                    for grp in range(NCH // 4):
                        pm = ps_main.tile([128, 4, 2 * S_OUT], F32, tag="pm")
                        for k in range(4):
                            ck = grp * 4 + k
                            nc.tensor.matmul(
                                pm[:, k, :],
                                xa[:, ck * 128:(ck + 1) * 128],
                                M2[:, b, :],
                                start=True, stop=True,
                            )
                        ys = y_sb[:, grp * 4:(grp + 1) * 4, :]
                        # block0 = min(yl+yr, 1); block1 = max(yl+yr-1, 0)
                        # block2 = max(yl-yr, 0); block3 = max(yr-yl, 0)
                        nc.vector.tensor_scalar_min(
                            ys[:, :, 0:32], pm[:, :, 0:32], 1.0
                        )
                        nc.vector.tensor_scalar(
                            out=ys[:, :, 32:64], in0=pm[:, :, 0:32],
                            scalar1=1.0, scalar2=0.0,
                            op0=OP.subtract, op1=OP.max,
                        )
                        nc.vector.tensor_scalar_max(
                            ys[:, :, 64:96], pm[:, :, 32:64], 0.0
                        )
                        nc.vector.tensor_scalar(
                            out=ys[:, :, 96:128], in0=pm[:, :, 32:64],
                            scalar1=-1.0, scalar2=0.0,
                            op0=OP.mult, op1=OP.max,
                        )
                    nc.sync.dma_start(
                        y_d[b, t4 * XCOLS:(t4 + 1) * XCOLS, :].rearrange(
                            "(c p) f -> p c f", p=128
                        ),
                        y_sb[:],
                    )

            # ---- P9: MLP (v_encode) ----
            with tc.tile_pool(name="ps_mlp", bufs=2, space="PSUM") as ps_mlp:
                h_sb = sb.tile([128, 2, BPC], F32)
                for jc in range(2):
                    ph = ps_mlp.tile([128, BPC], F32, tag="mlp")
                    for kc in range(Q // 128):
                        nc.tensor.matmul(
                            ph[:],
                            w1_sb[:, kc, jc * 128:(jc + 1) * 128],
                            VT[:, kc, :],
                            start=(kc == 0), stop=(kc == Q // 128 - 1),
                        )
                    nc.scalar.activation(
                        h_sb[:, jc, :], ph[:], AF.Lrelu,
                        bias=b1_sb[:, jc:jc + 1], scale=1.0, alpha=0.01,
                    )
                ve_sb = sb.tile([128, 2, BPC], F32)
                for j2c in range(2):
                    pv = ps_mlp.tile([128, BPC], F32, tag="mlp")
                    for kc in range(2):
                        nc.tensor.matmul(
                            pv[:],
                            w2_sb[:, kc, j2c * 128:(j2c + 1) * 128],
                            h_sb[:, kc, :],
                            start=(kc == 0), stop=(kc == 1),
                        )
                    nc.scalar.activation(
                        ve_sb[:, j2c, :], pv[:], AF.Identity,
                        bias=b2_sb[:, j2c:j2c + 1],
                    )
                nc.sync.dma_start(
                    ve_d.rearrange("(c p) b -> p c b", p=128), ve_sb[:]
                )

    if not nc.is_finalized():
        nc.finalize()
    return nc


def kernel(x, latent_vec, gumbel_u, K_left, K_right, temp, W1, b1, W2, b2):
    global LAST_RESULTS
    x = np.ascontiguousarray(x, np.float32)
    latent_vec = np.ascontiguousarray(latent_vec, np.float32)
    gumbel_u = np.ascontiguousarray(gumbel_u, np.float32)

    # host layout prep (pure layout/dtype, no math)
    if MM_DTYPE == "bf16":
        import ml_dtypes
        xT = np.ascontiguousarray(x.transpose(0, 2, 1).astype(ml_dtypes.bfloat16))
    else:
        xT = np.ascontiguousarray(x.transpose(0, 2, 1))          # [32, 64, 8192]
    Kcat = np.ascontiguousarray(
        np.stack([K_left.transpose(0, 2, 1), K_right.transpose(0, 2, 1)], axis=1)
    ).reshape(L, Q).astype(np.float32)
    latT = np.ascontiguousarray(latent_vec.T)                     # [256, 32]
    guTall = np.ascontiguousarray(gumbel_u[:, :, 0].transpose(2, 1, 3, 0))  # [64,2,32,32]
    W1perm = np.ascontiguousarray(
        np.asarray(W1, np.float32)
        .reshape(2, S_IN, S_OUT, L).transpose(0, 2, 1, 3).reshape(Q, L)
    )
    W2c = np.ascontiguousarray(np.asarray(W2, np.float32))
    b1c = np.ascontiguousarray(np.asarray(b1, np.float32))
    b2c = np.ascontiguousarray(np.asarray(b2, np.float32))
    t_clip = float(np.clip(np.float32(np.asarray(temp).reshape(-1)[0]), EPS, 2.0))
    p_exp = 1.0 / max(t_clip, EPS)

    key = (round(p_exp, 9), MM_DTYPE, LOGITS_DTYPE)
    if key not in _PROGRAM_CACHE:
        _PROGRAM_CACHE[key] = _build_program(p_exp)
    nc = _PROGRAM_CACHE[key]

    in_maps = []
    for core in range(N_CORES):
        b0 = core * BPC
        in_maps.append({
            "xT": np.ascontiguousarray(xT[b0:b0 + BPC]),
            "kcat": Kcat,
            "latT": np.ascontiguousarray(latT[:, b0:b0 + BPC]),
            "guT": np.ascontiguousarray(guTall[..., b0:b0 + BPC]),
            "w1": W1perm,
            "w2": W2c,
            "b1v": b1c,
            "b2v": b2c,
        })

    trace = bool(int(os.environ.get("CSG_TRACE", "0")))
    res = run_bass_kernel_spmd(
        nc, in_maps, core_ids=list(range(N_CORES)), trace=trace
    )
    LAST_RESULTS = res
    y = np.concatenate([r["y"] for r in res.results], axis=0)
    ve = np.concatenate([r["ve"].T for r in res.results], axis=0)
    return y.astype(np.float32), ve.astype(np.float32)


# revision 17
# speedup vs baseline: 1.3413x; 1.3413x over previous
"""Trainium2 Bass kernel for nn_CSG_layer (CSG layer: latent-conditioned softmax
mixing + gumbel routing + batched GEMM + tiny MLP side output).

Contract: kernel(**inputs) takes FULL unsharded inputs (as produced by
setup_inputs) and returns the full outputs (y, v_encode). Internally shards
batch dim 32 across 8 NeuronCores (4 batches/core), runs one SPMD Bass
program, and gathers.

Self-contained: only imports environment-provided packages (numpy, concourse).
"""
import os
import numpy as np
from contextlib import ExitStack

import concourse.bass as bass
import concourse.bacc as bacc
import concourse.tile as tile
from concourse import mybir
from concourse.bass_utils import run_bass_kernel_spmd
from concourse.masks import make_identity

F32 = mybir.dt.float32
BF16 = mybir.dt.bfloat16
EPS = float(np.finfo(np.float32).eps)

# Problem shape constants (fixed by the problem spec).
B, P, S_IN, S_OUT, L = 32, 8192, 64, 32, 256
N_CORES = 8
BPC = B // N_CORES          # batches per core = 4
Q = 2 * S_OUT * S_IN        # 4096 flattened (side, o, i)

# Main-GEMM compute dtype: "f32" (exact) or "bf16" (fast).
MM_DTYPE = os.environ.get("CSG_MM_DTYPE", "bf16")
# Logits matmul dtype: "f32" or "f32r"
LOGITS_DTYPE = os.environ.get("CSG_LOGITS_DTYPE", "f32r")

# Stash of the last BassKernelResults (for test.py profiling access).
LAST_RESULTS = None

_PROGRAM_CACHE = {}


def _build_program(p_exp: float):
    """Build the SPMD Bass program (same on every core)."""
    nc = bacc.Bacc()

    x_mm_dt = BF16 if MM_DTYPE == "bf16" else F32

    # ---- DRAM I/O (per-core shard shapes) ----
    logit_dt = mybir.dt.float32r if LOGITS_DTYPE == "f32r" else F32
    xT_d = nc.dram_tensor("xT", [BPC, S_IN, P], x_mm_dt, kind="ExternalInput")
    kcat_d = nc.dram_tensor("kcat", [L, Q], logit_dt, kind="ExternalInput")
    latT_d = nc.dram_tensor("latT", [L, BPC], logit_dt, kind="ExternalInput")
    guT_d = nc.dram_tensor("guT", [S_IN, 2, S_OUT, BPC], F32, kind="ExternalInput")
    w1_d = nc.dram_tensor("w1", [Q, L], F32, kind="ExternalInput")
    w2_d = nc.dram_tensor("w2", [L, L], F32, kind="ExternalInput")
    b1_d = nc.dram_tensor("b1v", [L], F32, kind="ExternalInput")
    b2_d = nc.dram_tensor("b2v", [L], F32, kind="ExternalInput")
    y_d = nc.dram_tensor("y", [BPC, P, 4 * S_OUT], F32, kind="ExternalOutput")
    ve_d = nc.dram_tensor("ve", [L, BPC], F32, kind="ExternalOutput")

    AF = mybir.ActivationFunctionType
    OP = mybir.AluOpType

    with tile.TileContext(nc) as tc, ExitStack() as ctx:
        consts = ctx.enter_context(tc.tile_pool(name="consts", bufs=1))
        sb = ctx.enter_context(tc.tile_pool(name="sb", bufs=1))

        ident = consts.tile([128, 128], F32)
        make_identity(nc, ident)
        ones64 = consts.tile([64, 1], F32)
        nc.vector.memset(ones64, 1.0)
        eps_sb = consts.tile([128, 1], F32)
        nc.vector.memset(eps_sb, EPS)

        # ---- small input DMAs ----
        lat_sb = sb.tile([128, 2, BPC], logit_dt)
        nc.sync.dma_start(lat_sb, latT_d.rearrange("(c p) b -> p c b", p=128))
        kcat_sb = sb.tile([128, 2, Q], logit_dt)
        nc.sync.dma_start(kcat_sb, kcat_d.rearrange("(c p) q -> p c q", p=128))
        gu_sb = sb.tile([S_IN, 2, S_OUT, BPC], F32)
        nc.sync.dma_start(gu_sb, guT_d[:])
        w1_sb = sb.tile([128, Q // 128, L], F32)
        nc.sync.dma_start(w1_sb, w1_d.rearrange("(c p) j -> p c j", p=128))
        w2_sb = sb.tile([128, 2, L], F32)
        nc.sync.dma_start(w2_sb, w2_d.rearrange("(c p) j -> p c j", p=128))
        b1_sb = sb.tile([128, 2], F32)
        nc.sync.dma_start(b1_sb, b1_d.rearrange("(c p) -> p c", p=128))
        b2_sb = sb.tile([128, 2], F32)
        nc.sync.dma_start(b2_sb, b2_d.rearrange("(c p) -> p c", p=128))

        # ---- P1/P2: logits -> E = exp(logits)  (layout [BPC, (side,o,i)]) ----
        E_sb = sb.tile([BPC, Q], F32)
        lat_mm = lat_sb[:]
        kcat_mm = kcat_sb[:]
        with tc.tile_pool(name="ps_log", bufs=2, space="PSUM") as ps_log:
            for qc in range(4):  # chunks of 1024
                pt_log = ps_log.tile([BPC, 1024], F32)
                for half in range(2):
                    lo = qc * 1024 + half * 512
                    for lc in range(2):
                        nc.tensor.matmul(
                            pt_log[:, half * 512:(half + 1) * 512],
                            lat_mm[:, lc, :],
                            kcat_mm[:, lc, lo:lo + 512],
                            start=(lc == 0),
                            stop=(lc == 1),
                        )
                nc.scalar.activation(
                    E_sb[:, qc * 1024:(qc + 1) * 1024], pt_log[:], AF.Exp
                )

        # S[b, g] = sum_i E ; A = E + EPS*S ; lnA = ln(A) ; V = E/S
        S_sb = sb.tile([BPC, 64], F32)
        nc.vector.tensor_reduce(
            S_sb[:], E_sb[:].rearrange("p (g i) -> p g i", i=S_IN),
            axis=mybir.AxisListType.X, op=OP.add,
        )
        A_sb = sb.tile([BPC, Q], F32)
        nc.vector.scalar_tensor_tensor(
            out=A_sb[:].rearrange("p (g i) -> p g i", i=S_IN),
            in0=S_sb[:, :, None].broadcast_to([BPC, 64, S_IN]),
            scalar=EPS,
            in1=E_sb[:].rearrange("p (g i) -> p g i", i=S_IN),
            op0=OP.mult,
            op1=OP.add,
        )
        lnA_sb = sb.tile([BPC, Q], F32)
        nc.scalar.activation(lnA_sb[:], A_sb[:], AF.Ln)
        recipS = sb.tile([BPC, 64], F32)
        nc.vector.reciprocal(recipS[:], S_sb[:])
        V_sb = sb.tile([BPC, Q], F32)
        nc.vector.tensor_mul(
            V_sb[:].rearrange("p (g i) -> p g i", i=S_IN),
            E_sb[:].rearrange("p (g i) -> p g i", i=S_IN),
            recipS[:, :, None].broadcast_to([BPC, 64, S_IN]),
        )

        # ---- P3: PE transposes ----
        # lnAT[i, side, o, b] ; VT[kpart, kc, b]
        lnAT = sb.tile([S_IN, 2, S_OUT, BPC], F32)
        VT = sb.tile([128, Q // 128, BPC], F32)
        with tc.tile_pool(name="ps_tr", bufs=4, space="PSUM") as ps_tr:
            for g in range(64):  # g = side*32 + o
                pt = ps_tr.tile([128, BPC], F32)
                nc.tensor.transpose(
                    pt[:S_IN, :], lnA_sb[:, g * S_IN:(g + 1) * S_IN],
                    ident[:BPC, :BPC],
                )
                nc.any.tensor_copy(lnAT[:, g // 32, g % 32, :], pt[:S_IN, :])
            for kc in range(Q // 128):
                pt = ps_tr.tile([128, BPC], F32)
                nc.tensor.transpose(
                    pt[:], V_sb[:, kc * 128:(kc + 1) * 128], ident[:BPC, :BPC]
                )
                nc.any.tensor_copy(VT[:, kc, :], pt[:])

            # ---- P4: gumbel chain in T layout [S_IN, (side,o,b)] ----
            NF = 2 * S_OUT * BPC  # 256
            uc = sb.tile([S_IN, NF], F32)
            nc.vector.tensor_scalar_max(uc[:], gu_sb[:].rearrange("p a b c -> p (a b c)"), EPS)
            lnu = sb.tile([S_IN, NF], F32)
            nc.scalar.activation(lnu[:], uc[:], AF.Ln)
            lnB = sb.tile([S_IN, NF], F32)
            nc.scalar.activation(
                lnB[:], lnu[:], AF.Ln, scale=-1.0, bias=eps_sb[:S_IN, :]
            )
            dch = sb.tile([S_IN, NF], F32)
            nc.vector.tensor_sub(
                dch[:], lnAT[:].rearrange("p a b c -> p (a b c)"), lnB[:]
            )
            ngum = sb.tile([S_IN, NF], F32)
            nc.scalar.activation(ngum[:], dch[:], AF.Exp, scale=float(p_exp))

            # ---- P5/P6: group sums over i (partition dim) via ones-matmul ----
            sg_ps = ps_tr.tile([1, NF], F32, tag="sg")
            nc.tensor.matmul(sg_ps[:], ones64[:], ngum[:], start=True, stop=True)
            recipG = sb.tile([1, NF], F32)
            nc.vector.reciprocal(recipG[:], sg_ps[:])
        repG = sb.tile([S_IN, NF], F32)
        nc.gpsimd.partition_broadcast(repG[:], recipG[:])
        maskT = sb.tile([S_IN, 2, S_OUT, BPC], F32)
        nc.vector.tensor_mul(
            maskT[:].rearrange("p a b c -> p (a b c)"),
            ngum[:],
            repG[:],
        )

        # ---- P7: per-batch moving operand M2 [64, (b, 2*S_OUT)] ----
        # cols 0:32 = L+R, 32:64 = L-R
        m2_dt = BF16 if MM_DTYPE == "bf16" else F32
        M2 = sb.tile([S_IN, BPC, 2 * S_OUT], m2_dt)
        for b in range(BPC):
            nc.vector.tensor_add(
                M2[:, b, 0:S_OUT], maskT[:, 0, :, b], maskT[:, 1, :, b]
            )
            nc.vector.tensor_sub(
                M2[:, b, S_OUT:2 * S_OUT], maskT[:, 0, :, b], maskT[:, 1, :, b]
            )

        # ---- P8: main GEMM + clip + y out ----
        # per batch: 4 xa tiles of [64, 2048]; 16 chunks of 128 points each
        XCOLS = 2048
        NCH = XCOLS // 128       # 16 chunks per xa tile
        xpool = ctx.enter_context(tc.tile_pool(name="xpool", bufs=2))
        ypool = ctx.enter_context(tc.tile_pool(name="ypool", bufs=2))
        with tc.tile_pool(name="ps_main", bufs=6, space="PSUM") as ps_main:
            for b in range(BPC):
                for t4 in range(P // XCOLS):
                    xa = xpool.tile([S_IN, XCOLS], x_mm_dt, tag="xa")
                    nc.sync.dma_start(
                        xa[:], xT_d[b, :, t4 * XCOLS:(t4 + 1) * XCOLS]
                    )
                    y_sb = ypool.tile([128, NCH, 4 * S_OUT], F32, tag="ysb")
                    for grp in range(NCH // 4):
                        pm = ps_main.tile([128, 4, 2 * S_OUT], F32, tag="pm")
                        for k in range(4):
                            ck = grp * 4 + k
                            nc.tensor.matmul(
                                pm[:, k, :],
                                xa[:, ck * 128:(ck + 1) * 128],
                                M2[:, b, :],
                                start=True, stop=True,
                            )
                        ys = y_sb[:, grp * 4:(grp + 1) * 4, :]
                        # block0 = min(yl+yr, 1); block1 = max(yl+yr-1, 0)
                        # block2 = max(yl-yr, 0); block3 = max(yr-yl, 0)
                        # blocks 0/1 on DVE; blocks 2/3 on ScalarE (Relu)
                        nc.vector.tensor_scalar_min(
                            ys[:, :, 0:32], pm[:, :, 0:32], 1.0
                        )
                        nc.vector.tensor_scalar(
                            out=ys[:, :, 32:64], in0=pm[:, :, 0:32],
                            scalar1=1.0, scalar2=0.0,
                            op0=OP.subtract, op1=OP.max,
                        )
                        nc.scalar.activation(
                            ys[:, :, 64:96], pm[:, :, 32:64], AF.Relu
                        )
                        nc.scalar.activation(
                            ys[:, :, 96:128], pm[:, :, 32:64], AF.Relu,
                            scale=-1.0,
                        )
                    nc.sync.dma_start(
                        y_d[b, t4 * XCOLS:(t4 + 1) * XCOLS, :].rearrange(
                            "(c p) f -> p c f", p=128
                        ),
                        y_sb[:],
                    )

            # ---- P9: MLP (v_encode) ----
            with tc.tile_pool(name="ps_mlp", bufs=2, space="PSUM") as ps_mlp:
                h_sb = sb.tile([128, 2, BPC], F32)
                for jc in range(2):
                    ph = ps_mlp.tile([128, BPC], F32, tag="mlp")
                    for kc in range(Q // 128):
                        nc.tensor.matmul(
                            ph[:],
                            w1_sb[:, kc, jc * 128:(jc + 1) * 128],
                            VT[:, kc, :],
                            start=(kc == 0), stop=(kc == Q // 128 - 1),
                        )
                    nc.scalar.activation(
                        h_sb[:, jc, :], ph[:], AF.Lrelu,
                        bias=b1_sb[:, jc:jc + 1], scale=1.0, alpha=0.01,
                    )
                ve_sb = sb.tile([128, 2, BPC], F32)
                for j2c in range(2):
                    pv = ps_mlp.tile([128, BPC], F32, tag="mlp")
                    for kc in range(2):
                        nc.tensor.matmul(
                            pv[:],
                            w2_sb[:, kc, j2c * 128:(j2c + 1) * 128],
                            h_sb[:, kc, :],
                            start=(kc == 0), stop=(kc == 1),
                        )
                    nc.scalar.activation(
                        ve_sb[:, j2c, :], pv[:], AF.Identity,
                        bias=b2_sb[:, j2c:j2c + 1],
                    )
                nc.sync.dma_start(
                    ve_d.rearrange("(c p) b -> p c b", p=128), ve_sb[:]
                )

    if not nc.is_finalized():
        nc.finalize()
    return nc


def kernel(x, latent_vec, gumbel_u, K_left, K_right, temp, W1, b1, W2, b2):
    global LAST_RESULTS
    x = np.ascontiguousarray(x, np.float32)
    latent_vec = np.ascontiguousarray(latent_vec, np.float32)
    gumbel_u = np.ascontiguousarray(gumbel_u, np.float32)

    # host layout prep (pure layout/dtype, no math)
    if MM_DTYPE == "bf16":
        import ml_dtypes
        xT = np.ascontiguousarray(x.transpose(0, 2, 1).astype(ml_dtypes.bfloat16))
    else:
        xT = np.ascontiguousarray(x.transpose(0, 2, 1))          # [32, 64, 8192]
    Kcat = np.ascontiguousarray(
        np.stack([K_left.transpose(0, 2, 1), K_right.transpose(0, 2, 1)], axis=1)
    ).reshape(L, Q).astype(np.float32)
    latT = np.ascontiguousarray(latent_vec.T)                     # [256, 32]
    guTall = np.ascontiguousarray(gumbel_u[:, :, 0].transpose(2, 1, 3, 0))  # [64,2,32,32]
    W1perm = np.ascontiguousarray(
        np.asarray(W1, np.float32)
        .reshape(2, S_IN, S_OUT, L).transpose(0, 2, 1, 3).reshape(Q, L)
    )
    W2c = np.ascontiguousarray(np.asarray(W2, np.float32))
    b1c = np.ascontiguousarray(np.asarray(b1, np.float32))
    b2c = np.ascontiguousarray(np.asarray(b2, np.float32))
    t_clip = float(np.clip(np.float32(np.asarray(temp).reshape(-1)[0]), EPS, 2.0))
    p_exp = 1.0 / max(t_clip, EPS)

    key = (round(p_exp, 9), MM_DTYPE, LOGITS_DTYPE)
    if key not in _PROGRAM_CACHE:
        _PROGRAM_CACHE[key] = _build_program(p_exp)
    nc = _PROGRAM_CACHE[key]

    in_maps = []
    for core in range(N_CORES):
        b0 = core * BPC
        in_maps.append({
            "xT": np.ascontiguousarray(xT[b0:b0 + BPC]),
            "kcat": Kcat,
            "latT": np.ascontiguousarray(latT[:, b0:b0 + BPC]),
            "guT": np.ascontiguousarray(guTall[..., b0:b0 + BPC]),
            "w1": W1perm,
            "w2": W2c,
            "b1v": b1c,
            "b2v": b2c,
        })

    trace = bool(int(os.environ.get("CSG_TRACE", "0")))
    res = run_bass_kernel_spmd(
        nc, in_maps, core_ids=list(range(N_CORES)), trace=trace
    )
    LAST_RESULTS = res
    y = np.concatenate([r["y"] for r in res.results], axis=0)
    ve = np.concatenate([r["ve"].T for r in res.results], axis=0)
    return y.astype(np.float32), ve.astype(np.float32)
